# revision 15
# baseline (speedup 1.0000x reference)
"""Multi-head attention (B=4, S=2048, D=768, H=12) on 8 Trainium2 cores.

Sharding: core c handles batch b=c//2 and heads [6*(c%2), 6*(c%2)+6).
Each core computes Q/K/V projections for its 6 heads (full sequence),
attention, and a partial out-projection (its 384 d_in columns of Wo).
Host gathers: out[b] = partial[2b] + partial[2b+1] + bo.

Schedule: the softmax exp stream on ScalarE is the critical path
(192 ACTIVATEs x ~1.15us = 220us floor); everything else is emitted as
a single global software-pipelined tick stream that keeps that stream
gapless and the PE HAM-warm. Per tick tau (one [128,2,512] exp):
lagged PV pair for tick tau-LAG, a budgeted quantum of filler matmuls
(Q/K/V projections, out-projection of finished q-chunks), the QK pair
for tick tau+2 (lookahead so exp never waits), then exp(tau). PSUM:
2x2-bank score supertiles (ping-pong), 2-bank PV accumulator (evicted
to SBUF by DVE right after the group's last PV so the next group's
PV can start), 2x1-bank projection accumulators. PE is pre-warmed with
dummy matmuls during the initial x/w DMA (4 queues, need-ordered
chunks) so the HAM clock gate is at 8/8 before the first projection.
Denominator: 65th all-ones V column accumulates sum(probs) as PV row
64; reciprocal via DMA-spread across partitions + GpSimd broadcast,
multiplied into the context eviction (all off the PE/ScalarE paths).
"""

import os
import numpy as np
import ml_dtypes

import concourse.bass as bass
import concourse.tile as tile
from concourse import bacc, mybir
from concourse import bass_utils

B, S, D, H = 4, 2048, 768, 12
HD = D // H          # 64
SCALE = HD ** -0.5   # 0.125
NCORES = 8
HPC = H // 2         # heads per core = 6
G = HPC // 2         # head-pair groups per core = 3
QC = S // 512        # query chunks of 512 = 4
KT = S // 128        # key tiles of 128 = 16
TT = S // 128        # token tiles = 16
KO = D // 128        # d_in k-tiles = 6
NG = QC * G          # 12 groups, qc-major: group = qc*G + g
NT = NG * KT         # 192 ticks

F32 = mybir.dt.float32
BF16 = mybir.dt.bfloat16
DT = BF16
NPDT = ml_dtypes.bfloat16

WARMUP_MM = 48       # dummy N=64 matmuls to lift the HAM clock gate
LAG = 4              # PV trails exp by LAG ticks
ENV = 1150.0         # ns of PE work budget per exp tick
CARRY_CAP = 1400.0   # max borrowed filler budget per tick

_CACHE = {}
LAST_RESULTS = None


def _patch_act_tables():
    """Steer every Exp/Ln activation to the one table set containing both,
    so the kernel does a single ACT_TABLE_LOAD."""
    from concourse import hw_specs
    orig = hw_specs.get_activation_tables

    def patched(arch):
        t = dict(orig(arch))
        both = {mybir.ActivationFunctionType.Exp, mybir.ActivationFunctionType.Ln}
        for name in t:
            if name != "natural_log_exp_and_others":
                t[name] = set(t[name]) - both
        return t

    bacc.get_activation_tables = patched


def build_nc():
    _patch_act_tables()
    nc = bacc.Bacc(None, target_bir_lowering=False, debug=False)

    xT_d = nc.dram_tensor("xT", [128, KO, S], DT, kind="ExternalInput")
    wq_d = nc.dram_tensor("wqT", [128, KO, HPC * HD], DT, kind="ExternalInput")
    wk_d = nc.dram_tensor("wkT", [128, KO, HPC * HD], DT, kind="ExternalInput")
    wv_d = nc.dram_tensor("wvT", [128, KO, HPC * HD], DT, kind="ExternalInput")
    wo_d = nc.dram_tensor("woT", [128, G, D], DT, kind="ExternalInput")
    bq_d = nc.dram_tensor("bq", [128, G], F32, kind="ExternalInput")
    bk_d = nc.dram_tensor("bk", [128, G], F32, kind="ExternalInput")
    bv_d = nc.dram_tensor("bv", [128, HPC * HD], F32, kind="ExternalInput")
    out_d = nc.dram_tensor("out", [128, TT, D], F32, kind="ExternalOutput")

    with tile.TileContext(nc) as tc:
        with (
            tc.tile_pool(name="consts", bufs=1) as consts,
            tc.tile_pool(name="acts", bufs=1) as acts,
            tc.tile_pool(name="probs", bufs=14) as probs_pool,
            tc.tile_pool(name="small", bufs=2) as small,
            tc.tile_pool(name="ctxp", bufs=4) as ctxp,
            tc.tile_pool(name="ostage", bufs=8) as ostage_pool,
            tc.tile_pool(name="pp", bufs=2, space="PSUM") as pp,
            tc.tile_pool(name="scores", bufs=2, space="PSUM") as scores_pool,
            tc.tile_pool(name="ctxps", bufs=1, space="PSUM") as ctx_pool,
        ):
            # ---------------- SBUF residents ----------------
            wk = consts.tile([128, KO, HPC * HD], DT)
            wq = consts.tile([128, KO, HPC * HD], DT)
            wv = consts.tile([128, KO, HPC * HD], DT)
            wo = consts.tile([128, G, D], DT)
            bk = consts.tile([128, G], F32)
            bq = consts.tile([128, G], F32)
            bv = consts.tile([128, HPC * HD], F32)
            xT = consts.tile([128, KO, S], DT)
            junk = consts.tile([128, 64], DT)

            qt = acts.tile([128, G, S], DT)   # feature-major Q^T
            kt = acts.tile([128, G, S], DT)   # feature-major K^T
            # token-major V, 65 cols per head: col 64 = 1.0 so each PV
            # matmul's 65th output row accumulates the softmax denominator
            vt = acts.tile([128, TT, HPC, HD + 1], DT)
            nc.vector.memset(junk[:], 0.0)
            nc.vector.memset(vt[:, :, :, HD:HD + 1], 1.0)

            # ---------------- input DMA: need-ordered, 4 queues ----------
            dq = [nc.sync, nc.gpsimd]
            dmai = [0]

            def load(dst, src):
                dq[dmai[0] % 2].dma_start(out=dst, in_=src)
                dmai[0] += 1

            gsl = lambda g_: slice(g_ * 128, (g_ + 1) * 128)
            nc.scalar.dma_start(out=wk[:, :, gsl(0)], in_=wk_d[:, :, gsl(0)])
            nc.scalar.dma_start(out=wq[:, :, gsl(0)], in_=wq_d[:, :, gsl(0)])
            for ko in range(KO):                       # x columns 0:512
                load(xT[:, ko, 0:512], xT_d[:, ko, 0:512])
            load(bk[:], bk_d[:])
            load(bq[:], bq_d[:])
            for ko in range(KO):                       # V weights
                load(wv[:, ko, :], wv_d[:, ko, :])
            load(bv[:], bv_d[:])
            for ko in range(KO):
                load(xT[:, ko, 512:1024], xT_d[:, ko, 512:1024])
            load(wk[:, :, gsl(1)], wk_d[:, :, gsl(1)])
            load(wq[:, :, gsl(1)], wq_d[:, :, gsl(1)])
            for ko in range(KO):
                load(xT[:, ko, 1024:1536], xT_d[:, ko, 1024:1536])
            load(wk[:, :, gsl(2)], wk_d[:, :, gsl(2)])
            load(wq[:, :, gsl(2)], wq_d[:, :, gsl(2)])
            for ko in range(KO):
                load(xT[:, ko, 1536:2048], xT_d[:, ko, 1536:2048])
            for g_ in range(G):
                load(wo[:, g_, :], wo_d[:, g_, :])

            # ---------------- PE warm-up (during DMA) ----------------
            wps = pp.tile([128, 512], F32, tag="pp")
            for _ in range(WARMUP_MM):
                nc.tensor.matmul(wps[0:64, 0:64], lhsT=junk[:], rhs=junk[:],
                                 start=True, stop=True)

            # ---------------- filler chain builders ----------------
            def proj_chain(w, b, dst, g_, c):
                """K or Q projection for one 512-col chunk: 6 accumulating
                matmuls + DVE bias-add eviction."""
                st = {}

                def mk(ko):
                    def f():
                        if ko == 0:
                            st["ps"] = pp.tile([128, 512], F32, tag="pp", name="pjps")
                        nc.tensor.matmul(
                            st["ps"][:],
                            lhsT=w[:, ko, gsl(g_)],
                            rhs=xT[:, ko, c * 512:(c + 1) * 512],
                            start=(ko == 0), stop=(ko == KO - 1),
                        )
                    return f

                steps = [(225.0, mk(ko)) for ko in range(KO)]

                def evict():
                    nc.vector.tensor_scalar_add(
                        out=dst[:, g_, c * 512:(c + 1) * 512],
                        in0=st["ps"][:], scalar1=b[:, g_:g_ + 1])
                steps.append((15.0, evict))
                return steps

            def v_chain(tt):
                st = {}

                def mk(ko):
                    def f():
                        if ko == 0:
                            st["ps"] = pp.tile([128, 512], F32, tag="pp", name="pjps")
                        nc.tensor.matmul(
                            st["ps"][:, 0:HPC * HD],
                            lhsT=xT[:, ko, tt * 128:(tt + 1) * 128],
                            rhs=wv[:, ko, :],
                            start=(ko == 0), stop=(ko == KO - 1),
                        )
                    return f

                steps = [(175.0, mk(ko)) for ko in range(KO)]

                def evict():
                    nc.vector.tensor_add(
                        out=vt[:, tt, :, 0:HD],
                        in0=st["ps"][:, 0:HPC * HD].rearrange(
                            "p (h d) -> p h d", h=HPC),
                        in1=bv[:].rearrange("p (h d) -> p h d", h=HPC))
                steps.append((15.0, evict))
                return steps

            ctx_t_by_qc = {}

            def oproj_chain(qc, tl, dmaq):
                st = {}
                steps = []
                for nh in range(2):
                    def mk(nh_, g2):
                        def f():
                            if g2 == 0:
                                st[nh_] = pp.tile([128, 384], F32, tag="pp", name="pops")
                            nc.tensor.matmul(
                                st[nh_][:],
                                lhsT=ctx_t_by_qc[qc][:, g2, tl * 128:(tl + 1) * 128],
                                rhs=wo[:, g2, nh_ * 384:(nh_ + 1) * 384],
                                start=(g2 == 0), stop=(g2 == G - 1),
                            )
                        return f
                    steps += [(175.0, mk(nh, g2)) for g2 in range(G)]

                    def cp(nh_):
                        def f():
                            if nh_ == 0:
                                st["ost"] = ostage_pool.tile([128, D], F32, name="ost")
                            nc.vector.tensor_copy(
                                out=st["ost"][:, nh_ * 384:(nh_ + 1) * 384],
                                in_=st[nh_][:])
                        return f
                    steps.append((15.0, cp(nh)))

                def dma():
                    dmaq.dma_start(out=out_d[:, qc * 4 + tl, :], in_=st["ost"][:])
                steps.append((15.0, dma))
                return steps

            # ---------------- filler queue (deadline order) ----------
            den_emitted = set()

            class Chain:
                __slots__ = ("steps", "i", "gate")

                def __init__(self, steps, gate=None):
                    self.steps, self.i, self.gate = steps, 0, gate

            fq = []
            chains = {}   # cid -> Chain, for emission-order prerequisites
            oq = [nc.sync, nc.sync]

            def add(steps, gate=None, cid=None):
                ch = Chain(steps, gate)
                fq.append(ch)
                if cid is not None:
                    chains[cid] = ch

            # g-major deadline order: groups run [(g0,qc0..3), (g1,*),
            # (g2,*)], so K/Q/V projection demand spreads over the whole g0
            # phase instead of crunching into the first two groups.
            add(v_chain(0), cid="v0"); add(v_chain(1), cid="v1")
            add(proj_chain(wk, bk, kt, 0, 1), cid="k0c1")
            add(v_chain(2), cid="v2"); add(v_chain(3), cid="v3")
            add(proj_chain(wk, bk, kt, 0, 2), cid="k0c2")
            add(v_chain(4), cid="v4"); add(v_chain(5), cid="v5")
            add(proj_chain(wk, bk, kt, 0, 3), cid="k0c3")
            add(v_chain(6), cid="v6"); add(v_chain(7), cid="v7")
            add(v_chain(8), cid="v8")
            add(proj_chain(wq, bq, qt, 0, 1), cid="q0qc1")
            for tt in range(9, TT):
                add(v_chain(tt), cid=f"v{tt}")
            add(proj_chain(wq, bq, qt, 0, 2), cid="q0qc2")
            add(proj_chain(wq, bq, qt, 0, 3), cid="q0qc3")
            otail_by_tl = {}

            def oproj_pass(tl, g2):
                """qc3 out-projection, one g-pass: 2 matmuls + DVE
                copy/accumulate into a persistent SBUF stage, so only the
                final 2-matmul pass (per tl) trails the last softmax."""
                st = {}
                steps = []
                for nh in range(2):
                    def mm(nh_, g2_):
                        def f():
                            st[nh_] = pp.tile([128, 384], F32, tag="pp",
                                              name="pot")
                            nc.tensor.matmul(
                                st[nh_][:],
                                lhsT=ctx_t_by_qc[QC - 1][:, g2_,
                                                         tl * 128:(tl + 1) * 128],
                                rhs=wo[:, g2_, nh_ * 384:(nh_ + 1) * 384],
                                start=True, stop=True)
                        return f
                    steps.append((175.0, mm(nh, g2)))

                    def ev(nh_, g2_):
                        def f():
                            o = otail_by_tl.get(tl)
                            if o is None:
                                o = ostage_pool.tile([128, D], F32, name="otl")
                                otail_by_tl[tl] = o
                            dst = o[:, nh_ * 384:(nh_ + 1) * 384]
                            if g2_ == 0:
                                nc.vector.tensor_copy(out=dst, in_=st[nh_][:])
                            else:
                                nc.vector.tensor_add(out=dst, in0=dst,
                                                     in1=st[nh_][:])
                        return f
                    steps.append((15.0, ev(nh, g2)))
                if g2 == G - 1:
                    tq = [nc.scalar, nc.sync, nc.gpsimd]

                    def dma():
                        tq[tl % 3].dma_start(
                            out=out_d[:, (QC - 1) * 4 + tl, :],
                            in_=otail_by_tl[tl][:])
                    steps.append((15.0, dma))
                return steps
            for tl in range(4):
                add(oproj_pass(tl, 0),
                    gate=(lambda: (QC - 1) in den_emitted))
            add(proj_chain(wk, bk, kt, 1, 0), cid="k1c0")
            add(proj_chain(wq, bq, qt, 1, 0), cid="q1qc0")
            for c in range(1, QC):
                add(proj_chain(wk, bk, kt, 1, c), cid=f"k1c{c}")
            add(proj_chain(wq, bq, qt, 1, 1), cid="q1qc1")
            add(proj_chain(wq, bq, qt, 1, 2), cid="q1qc2")
            add(proj_chain(wq, bq, qt, 1, 3), cid="q1qc3")
            for tl in range(4):
                add(oproj_pass(tl, 1),
                    gate=(lambda: (2 * QC - 1) in den_emitted))
            add(proj_chain(wk, bk, kt, 2, 0), cid="k2c0")
            add(proj_chain(wq, bq, qt, 2, 0), cid="q2qc0")
            for c in range(1, QC):
                add(proj_chain(wk, bk, kt, 2, c), cid=f"k2c{c}")
            for tl in range(4):
                add(oproj_chain(0, tl, oq[tl % 2]),
                    gate=(lambda: 2 * QC in den_emitted))
            add(proj_chain(wq, bq, qt, 2, 1), cid="q2qc1")
            for tl in range(4):
                add(oproj_chain(1, tl, oq[tl % 2]),
                    gate=(lambda: 2 * QC + 1 in den_emitted))
            add(proj_chain(wq, bq, qt, 2, 2), cid="q2qc2")
            for tl in range(4):
                add(oproj_chain(2, tl, oq[tl % 2]),
                    gate=(lambda: 2 * QC + 2 in den_emitted))
            add(proj_chain(wq, bq, qt, 2, 3), cid="q2qc3")
            for tl in range(4):
                add(oproj_pass(tl, 2),
                    gate=(lambda: (NG - 1) in den_emitted))


            carry = [0.0]

            def _first_open():
                for ch in fq:
                    if ch.gate is None or ch.gate():
                        return ch
                return None

            def _pop_one(ch):
                cost, fn = ch.steps[ch.i]
                fn()
                carry[0] -= cost
                ch.i += 1
                if ch.i == len(ch.steps):
                    fq.remove(ch)

            def run_filler():
                while fq:
                    ch = _first_open()
                    if ch is None or ch.steps[ch.i][0] > carry[0]:
                        return
                    _pop_one(ch)

            def ensure(cid):
                """Force-emit (borrowing budget) until chain `cid` is fully
                emitted, draining open chains queued ahead of it first."""
                ch = chains.get(cid)
                if ch is None:
                    return
                while ch.i < len(ch.steps):
                    nxt = _first_open()
                    assert nxt is not None, cid
                    _pop_one(nxt)

            # ---------------- attention stream ----------------
            st_by_tau = {}
            pr_by_tau = {}
            cps_by_group = {}

            def qk(tau):
                Gr, t2 = divmod(tau, 16)
                g_, qc = divmod(Gr, QC)
                ensure(f"k{g_}c{t2 // 4}")
                ensure(f"q{g_}qc{qc}")
                st_ = scores_pool.tile([128, 2, 512], F32, tag="st")
                st_by_tau[tau] = st_
                ks = slice(t2 * 128, (t2 + 1) * 128)
                qs = slice(qc * 512, (qc + 1) * 512)
                nc.tensor.matmul(st_[:, 0, :], lhsT=kt[0:64, g_, ks],
                                 rhs=qt[0:64, g_, qs], start=True, stop=True)
                nc.tensor.matmul(st_[:, 1, :], lhsT=kt[64:128, g_, ks],
                                 rhs=qt[64:128, g_, qs], start=True, stop=True)

            def expf(tau):
                pr = probs_pool.tile([128, 2, 512], DT, tag="pr", name="pr")
                pr_by_tau[tau] = pr
                nc.scalar.activation(
                    out=pr[:], in_=st_by_tau.pop(tau)[:],
                    func=mybir.ActivationFunctionType.Exp, scale=SCALE)

            def pv(Gp, t2p):
                ensure(f"v{t2p}")
                if t2p == 0:
                    cps_by_group[Gp] = ctx_pool.tile(
                        [128, 2, 512], F32, tag="ctx", name="cps")
                    if Gp >= 1:
                        del cps_by_group[Gp - 1]
                cps = cps_by_group[Gp]
                g_ = Gp // QC
                pr = pr_by_tau.pop(16 * Gp + t2p)
                for h in range(2):
                    nc.tensor.matmul(
                        cps[0:HD + 1, h, :],
                        lhsT=vt[:, t2p, 2 * g_ + h, :],
                        rhs=pr[:, h, :],
                        start=(t2p == 0), stop=(t2p == KT - 1),
                    )
                if t2p == KT - 1:
                    finish_group(Gp)

            def finish_group(Gp):
                """Evict the PV accumulator to SBUF (frees the psum bank for
                the next group), then reciprocal + normalize into ctx_t."""
                g_, qc = divmod(Gp, QC)
                cps = cps_by_group[Gp]
                stage = small.tile([128, 2, 512], F32, tag="cstage")
                nc.vector.tensor_copy(out=stage[0:HD + 1, :, :],
                                      in_=cps[0:HD + 1, :, :])
                if g_ == 0:
                    ctx_t_by_qc[qc] = ctxp.tile([128, G, 512], DT, name="ctxt")
                ctx_t = ctx_t_by_qc[qc]
                spread = small.tile([128, 8], F32, tag="spread")
                nc.gpsimd.dma_start(out=spread[:, :], in_=stage[64:65, :, :])
                rs = small.tile([128, 8], F32, tag="rspread")
                nc.vector.reciprocal(out=rs[:], in_=spread[:])
                rcp = small.tile([128, 2, 512], F32, tag="rcp")
                nc.gpsimd.dma_start(out=rcp[0:1, :, :], in_=rs[:, :])
                bc = small.tile([64, 2, 512], F32, tag="bc")
                nc.gpsimd.partition_broadcast(
                    out_ap=bc[0:64, :, :], in_ap=rcp[0:1, :, :], channels=64)
                nc.vector.tensor_mul(
                    out=ctx_t[0:64, g_, :], in0=stage[0:64, 0, :],
                    in1=bc[0:64, 0, :])
                stgB = small.tile([128, 512], DT, tag="stgB")
                nc.vector.tensor_mul(
                    out=stgB[0:64, :], in0=stage[0:64, 1, :], in1=bc[0:64, 1, :])
                nc.gpsimd.dma_start(out=ctx_t[64:128, g_, :], in_=stgB[0:64, :])
                den_emitted.add(Gp)

            # PV emission schedule: tick LAG+1 of each group emits that
            # group's t2=0 and t2=1 (the 1-tick delay lets the previous
            # group's psum eviction land); after that one pair per tick.
            def pv_for_tick(tau):
                Gr, t2 = divmod(tau, 16)
                out = []
                if t2 == LAG + 1:
                    out.append((Gr, 0))
                    out.append((Gr, 1))
                elif t2 > LAG + 1:
                    out.append((Gr, t2 - LAG))
                elif t2 <= LAG and Gr > 0 and (t2 + 16 - LAG) <= 15:
                    out.append((Gr - 1, t2 + 16 - LAG))
                return out

            # ---- lead-in: first K/Q chains + 2 lookahead QK pairs ----
            for step in proj_chain(wk, bk, kt, 0, 0):
                step[1]()
            for step in proj_chain(wq, bq, qt, 0, 0):
                step[1]()
            qk(0)
            qk(1)

            # ---- the tick stream ----
            for tau in range(NT):
                pvs = pv_for_tick(tau)
                cost = (213.0 if tau + 2 < NT else 0.0) + 426.0 * len(pvs)
                carry[0] = min(CARRY_CAP, carry[0] + ENV - cost)
                for Gp, t2p in pvs:
                    pv(Gp, t2p)
                run_filler()
                expf(tau)
                if tau + 2 < NT:
                    qk(tau + 2)

            # ---- tail: remaining PVs, then remaining filler ----
            for t2p in range(16 - LAG, 16):
                pv(NG - 1, t2p)
            carry[0] = 1e9
            while fq:
                run_filler()

    nc.compile()
    return nc


def _prep_inputs(x, Wq, bq, Wk, bk, Wv, bv, Wo):
    """Build the 8 per-core input maps (host-side shard + layout prep)."""
    def part_major(a):  # [(ko*128), m] -> [128, ko, m]
        k = a.shape[0] // 128
        return np.ascontiguousarray(
            a.reshape(k, 128, a.shape[1]).transpose(1, 0, 2))

    xT = [part_major(np.ascontiguousarray(x[b].T).astype(NPDT)) for b in range(B)]
    WqT, WkT, WvT = (np.ascontiguousarray(W.T.astype(NPDT)) for W in (Wq, Wk, Wv))
    WoT = np.ascontiguousarray(Wo.T.astype(NPDT))

    in_maps = []
    for c in range(NCORES):
        b = c // 2
        hs = (c % 2) * HPC * HD  # d slice start (384-wide)
        sl = slice(hs, hs + HPC * HD)
        in_maps.append({
            "xT": xT[b],
            "wqT": part_major(WqT[:, sl]),
            "wkT": part_major(WkT[:, sl]),
            "wvT": part_major(WvT[:, sl]),
            "woT": part_major(np.ascontiguousarray(WoT[sl, :])),
            "bq": np.ascontiguousarray(
                bq[sl].astype(np.float32).reshape(G, 128).T),
            "bk": np.ascontiguousarray(
                bk[sl].astype(np.float32).reshape(G, 128).T),
            "bv": np.ascontiguousarray(
                np.broadcast_to(bv[sl].astype(np.float32), (128, HPC * HD))),
        })
    return in_maps


def kernel(x, Wq, bq, Wk, bk, Wv, bv, Wo, bo):
    global LAST_RESULTS
    x, Wq, bq, Wk, bk, Wv, bv, Wo, bo = (
        np.asarray(a) for a in (x, Wq, bq, Wk, bk, Wv, bv, Wo, bo))
    if "nc" not in _CACHE:
        _CACHE["nc"] = build_nc()
    nc = _CACHE["nc"]
    in_maps = _prep_inputs(x, Wq, bq, Wk, bk, Wv, bv, Wo)
    res = bass_utils.run_bass_kernel_spmd(nc, in_maps, core_ids=list(range(NCORES)))
    LAST_RESULTS = res
    out = np.empty((B, S, D), np.float32)
    for b in range(B):
        p0 = res.results[2 * b]["out"].transpose(1, 0, 2).reshape(S, D)
        p1 = res.results[2 * b + 1]["out"].transpose(1, 0, 2).reshape(S, D)
        out[b] = p0 + p1 + bo.astype(np.float32)
    return out


if __name__ == "__main__":
    rng = np.random.default_rng(0)
    ins = {
        "x": rng.standard_normal((B, S, D), dtype=np.float32),
        "Wq": (rng.standard_normal((D, D), dtype=np.float32) * D ** -0.5),
        "Wk": (rng.standard_normal((D, D), dtype=np.float32) * D ** -0.5),
        "Wv": (rng.standard_normal((D, D), dtype=np.float32) * D ** -0.5),
        "Wo": (rng.standard_normal((D, D), dtype=np.float32) * D ** -0.5),
        "bq": rng.standard_normal(D, dtype=np.float32) * 0.01,
        "bk": rng.standard_normal(D, dtype=np.float32) * 0.01,
        "bv": rng.standard_normal(D, dtype=np.float32) * 0.01,
        "bo": rng.standard_normal(D, dtype=np.float32) * 0.01,
    }
    out = kernel(**ins)
    print("kernel ran, out:", out.shape, out.dtype, float(np.abs(out).mean()))
